# revision 2
# baseline (speedup 1.0000x reference)
"""Trainium2 Bass kernel for the stacked-attention module (8 NeuronCores).

Strategy: pure data parallel over batch (B=128 -> 16 batches/core).
All matmuls in bf16 (PE full rate), f32 accumulation in PSUM, f32 running
state for u.  Softmax over the spatial dim is computed without the max
subtraction (logits are tanh outputs in (-1,1), exp is safe) and without
normalizing p explicitly: u += (sum_s e*vi) / (sum_s e).

Host-side (untimed) preprocessing packs every tensor in the exact SBUF
layout so each DMA is a straight partition-major copy:
  vi   [pair, p, ct, 392]   bf16   (pair = 2 batches side by side in free dim)
  vq   [p, ht, b, t]        f32
  w1   [p, ct, m]           bf16   (= l1_w.T tiles: c = ct*128+p, h = m)
  wvi* [p, ht, k]           bf16   (= w_vi.T: h = ht*128+p)
  wu*  [p, ht, k]           bf16   (= w_u.T)
  l1b  [p, ht] f32, bu* [p, kt] f32
  out  [p, kt, b]           f32    (u transposed; host untransposes)
"""

import numpy as np
from ml_dtypes import bfloat16

import concourse.bass as bass
import concourse.tile as tile
from concourse import bacc, mybir
from concourse.bass import ts, ds
from concourse.bass_utils import run_bass_kernel_spmd

BF = mybir.dt.bfloat16
F32 = mybir.dt.float32

NCORES = 8
B = 128
C = 2048
S = 196          # 14*14 spatial
HID = 1024
T = 20
BL = B // NCORES  # 16 batches per core
NPAIR = BL // 2   # 8
CT = C // 128     # 16 contraction tiles for l1
HT = HID // 128   # 8 hidden tiles
S2 = 2 * S        # 392, two batches side by side

_NC = None


def _build():
    nc = bacc.Bacc(None)

    vi_p = nc.declare_dram_parameter("vi", [NPAIR, 128, CT, S2], BF, isOutput=False)
    vq_p = nc.declare_dram_parameter("vq", [128, HT, BL, T], F32, isOutput=False)
    w1_p = nc.declare_dram_parameter("w1", [128, CT, HID], BF, isOutput=False)
    wvi0_p = nc.declare_dram_parameter("wvi0", [128, HT, HID], BF, isOutput=False)
    wu0_p = nc.declare_dram_parameter("wu0", [128, HT, HID], BF, isOutput=False)
    wvi1_p = nc.declare_dram_parameter("wvi1", [128, HT, HID], BF, isOutput=False)
    wu1_p = nc.declare_dram_parameter("wu1", [128, HT, HID], BF, isOutput=False)
    l1b_p = nc.declare_dram_parameter("l1b", [128, HT], F32, isOutput=False)
    bu0_p = nc.declare_dram_parameter("bu0", [128, HT], F32, isOutput=False)
    bu1_p = nc.declare_dram_parameter("bu1", [128, HT], F32, isOutput=False)
    out_p = nc.declare_dram_parameter("out", [128, HT, BL], F32, isOutput=True)

    wvi_p = [wvi0_p, wvi1_p]
    wu_p = [wu0_p, wu1_p]
    bu_p = [bu0_p, bu1_p]

    Tanh = mybir.ActivationFunctionType.Tanh
    Exp = mybir.ActivationFunctionType.Exp
    X = mybir.AxisListType.X

    with tile.TileContext(nc) as tc:
        with (
            tc.tile_pool(name="weights", bufs=1) as wpool,
            tc.tile_pool(name="xin", bufs=2) as xpool,
            tc.tile_pool(name="vis", bufs=1) as vipool,
            tc.tile_pool(name="small", bufs=2) as spool,
            tc.tile_pool(name="uu", bufs=3) as upool,
            tc.tile_pool(name="act", bufs=4) as apool,
            tc.tile_pool(name="mm", bufs=4, space="PSUM") as mmpool,
            tc.tile_pool(name="vqtp", bufs=2, space="PSUM") as vqtpool,
        ):
            # ---- weights / vq loads (l1 weights on sync queue, hop weights
            # on gpsimd queue so they don't head-block the x streaming) ----
            w1_sb = wpool.tile([128, CT, HID], BF, tag="w1")
            nc.sync.dma_start(out=w1_sb[:], in_=w1_p[:])
            vq_sb = wpool.tile([128, HT, BL, T], F32, tag="vq")
            nc.sync.dma_start(out=vq_sb[:], in_=vq_p[:])
            l1b_sb = wpool.tile([128, HT], F32, tag="l1b")
            nc.sync.dma_start(out=l1b_sb[:], in_=l1b_p[:])

            wvi_sb = []
            wu_sb = []
            bu_sb = []
            for i in range(2):
                wv = wpool.tile([128, HT, HID], BF, tag=f"wvi{i}")
                nc.gpsimd.dma_start(out=wv[:], in_=wvi_p[i][:])
                wvi_sb.append(wv)
                wu = wpool.tile([128, HT, HID], BF, tag=f"wu{i}")
                nc.gpsimd.dma_start(out=wu[:], in_=wu_p[i][:])
                wu_sb.append(wu)
                bu = wpool.tile([128, HT], F32, tag=f"bu{i}")
                nc.gpsimd.dma_start(out=bu[:], in_=bu_p[i][:])
                bu_sb.append(bu)

            # ---- u0 = mean_t(v_q) : [p, ht, b] f32 ----
            u_cur = upool.tile([128, HT, BL], F32, tag="u")
            for ht in range(HT):
                nc.vector.reduce_sum(out=u_cur[:, ht, :], in_=vq_sb[:, ht, :, :], axis=X)
            nc.vector.tensor_scalar_mul(out=u_cur[:], in0=u_cur[:], scalar1=1.0 / T)

            # ---- layer 1: vi = tanh(x @ l1_w.T + l1_b), kept as [h, s] tiles ----
            vi_sb = vipool.tile([128, HT, NPAIR, S2], BF, tag="vi")
            for pair in range(NPAIR):
                x_sb = xpool.tile([128, CT, S2], BF, tag="x")
                nc.sync.dma_start(out=x_sb[:], in_=vi_p[pair])
                for ht in range(HT):
                    ps = mmpool.tile([128, S2], F32, tag="mm")
                    for ct in range(CT):
                        nc.tensor.matmul(
                            ps[:],
                            w1_sb[:, ct, ts(ht, 128)],
                            x_sb[:, ct, :],
                            start=(ct == 0),
                            stop=(ct == CT - 1),
                        )
                    nc.scalar.activation(
                        out=vi_sb[:, ht, pair, :],
                        in_=ps[:],
                        func=Tanh,
                        bias=l1b_sb[:, ht : ht + 1],
                        scale=1.0,
                    )

            # ---- two attention hops ----
            for hop in range(2):
                u_bf = spool.tile([128, HT, BL], BF, tag="ubf")
                nc.vector.tensor_copy(out=u_bf[:], in_=u_cur[:])

                # v_q_t[k, b] = sum_h w_u[k, h] u[h, b]  (+ b_u)
                vqt_ps = vqtpool.tile([128, HT, BL], F32, tag="vqt")
                for kt in range(HT):
                    for ht in range(HT):
                        nc.tensor.matmul(
                            vqt_ps[:, kt, :],
                            wu_sb[hop][:, ht, ts(kt, 128)],
                            u_bf[:, ht, :],
                            start=(ht == 0),
                            stop=(ht == HT - 1),
                        )
                vqt_sb = spool.tile([128, HT, BL], F32, tag="vqts")
                for kt in range(HT):
                    nc.vector.tensor_scalar(
                        out=vqt_sb[:, kt, :],
                        in0=vqt_ps[:, kt, :],
                        scalar1=bu_sb[hop][:, kt : kt + 1],
                        scalar2=None,
                        op0=mybir.AluOpType.add,
                    )

                r_sb = spool.tile([128, HT, BL], F32, tag="r")
                z_sb = spool.tile([128, HT, BL], F32, tag="z")
                for pair in range(NPAIR):
                    for kt in range(HT):
                        ps = mmpool.tile([128, S2], F32, tag="mm")
                        for ht in range(HT):
                            nc.tensor.matmul(
                                ps[:],
                                wvi_sb[hop][:, ht, ts(kt, 128)],
                                vi_sb[:, ht, pair, :],
                                start=(ht == 0),
                                stop=(ht == HT - 1),
                            )
                        ha = apool.tile([128, S2], BF, tag="ha")
                        for j in range(2):
                            b = 2 * pair + j
                            nc.scalar.activation(
                                out=ha[:, ds(S * j, S)],
                                in_=ps[:, ds(S * j, S)],
                                func=Tanh,
                                bias=vqt_sb[:, kt, b : b + 1],
                                scale=1.0,
                            )
                        e = apool.tile([128, S2], BF, tag="e")
                        nc.scalar.activation(out=e[:], in_=ha[:], func=Exp)
                        nc.vector.reduce_sum(
                            out=z_sb[:, kt, ts(pair, 2)],
                            in_=e[:].rearrange("p (j s) -> p j s", j=2),
                            axis=X,
                        )
                        tt = apool.tile([128, S2], BF, tag="tt")
                        nc.vector.tensor_mul(
                            out=tt[:], in0=e[:], in1=vi_sb[:, kt, pair, :]
                        )
                        nc.vector.reduce_sum(
                            out=r_sb[:, kt, ts(pair, 2)],
                            in_=tt[:].rearrange("p (j s) -> p j s", j=2),
                            axis=X,
                        )

                zr = spool.tile([128, HT, BL], F32, tag="zr")
                nc.vector.reciprocal(out=zr[:], in_=z_sb[:])
                upd = spool.tile([128, HT, BL], F32, tag="upd")
                nc.vector.tensor_mul(out=upd[:], in0=r_sb[:], in1=zr[:])
                u_next = upool.tile([128, HT, BL], F32, tag="u")
                nc.vector.tensor_add(out=u_next[:], in0=u_cur[:], in1=upd[:])
                u_cur = u_next

            nc.sync.dma_start(out=out_p[:], in_=u_cur[:])

    nc.compile()
    return nc


def _get_nc():
    global _NC
    if _NC is None:
        _NC = _build()
    return _NC


def _prep_in_maps(v_i, v_q, l1_w, l1_b, w_vi0, w_u0, b_u0, w_vi1, w_u1, b_u1):
    v_i = np.asarray(v_i, np.float32)
    v_q = np.asarray(v_q, np.float32)

    # vi: [B, C, H, W] -> [core, pair, p, ct, j, s] -> [core, pair, p, ct, 392]
    vib = v_i.reshape(B, C, S).astype(bfloat16)
    vib = vib.reshape(NCORES, NPAIR, 2, CT, 128, S).transpose(0, 1, 4, 3, 2, 5)
    vib = np.ascontiguousarray(vib).reshape(NCORES, NPAIR, 128, CT, S2)

    # vq: [B, T, HID] -> [core, p, ht, b, t]
    vq = v_q.reshape(NCORES, BL, T, HT, 128).transpose(0, 4, 3, 1, 2)
    vq = np.ascontiguousarray(vq)

    def packT(w, ntiles):
        # w [out, in] -> w.T [in, out] -> [p, tile, out] with in = tile*128+p
        wt = np.asarray(w, np.float32).T.astype(bfloat16)
        return np.ascontiguousarray(
            wt.reshape(ntiles, 128, w.shape[0]).transpose(1, 0, 2)
        )

    w1h = packT(l1_w, CT)           # [128, 16, 1024]
    wvi0h = packT(w_vi0, HT)        # [128, 8, 1024]
    wvi1h = packT(w_vi1, HT)
    wu0h = packT(w_u0, HT)
    wu1h = packT(w_u1, HT)

    def packb(b):
        return np.ascontiguousarray(np.asarray(b, np.float32).reshape(HT, 128).T)

    l1bh = packb(l1_b)
    bu0h = packb(b_u0)
    bu1h = packb(b_u1)

    in_maps = []
    for core in range(NCORES):
        in_maps.append(
            {
                "vi": vib[core],
                "vq": vq[core],
                "w1": w1h,
                "wvi0": wvi0h,
                "wu0": wu0h,
                "wvi1": wvi1h,
                "wu1": wu1h,
                "l1b": l1bh,
                "bu0": bu0h,
                "bu1": bu1h,
            }
        )
    return in_maps


def run_sharded(inputs: dict, trace: bool = False):
    """Returns (full_output [128,1024] f32, BassKernelResults)."""
    nc = _get_nc()
    in_maps = _prep_in_maps(**inputs)
    res = run_bass_kernel_spmd(
        nc, in_maps, core_ids=list(range(NCORES)), trace=trace
    )
    outs = []
    for i in range(NCORES):
        o = np.asarray(res.results[i]["out"])  # [128, HT, BL] = [p, ht, b]
        outs.append(np.ascontiguousarray(o.transpose(2, 1, 0)).reshape(BL, HID))
    full = np.concatenate(outs, axis=0).astype(np.float32)
    return full, res


def kernel(**inputs) -> np.ndarray:
    out, _ = run_sharded(inputs, trace=False)
    return out


# revision 6
# speedup vs baseline: 1.1627x; 1.1627x over previous
"""Trainium2 Bass kernel for the stacked-attention module (8 NeuronCores).

Pure data parallel over batch (B=128 -> 16 batches/core, processed as 8
pairs with the pair side-by-side in the matmul free dim).

Pipeline (per core):
  phase A: for each pair: l1 (bf16 matmuls, PE-heavy) immediately followed
           by hop0 for the same pair (fp8 DoubleRow matmuls + ACT/DVE
           softmax) -- hop0's ACT/DVE work hides under the next pair's l1.
  boundary: batched u1 = u0 + sum_s(e*vi)/sum_s(e); batched v_q_t for hop1.
  phase B: hop1 for all pairs.

Softmax over the spatial dim needs no max subtraction (logits are tanh
outputs in (-1,1)) and p is never normalized: u += (sum e*vi) / (sum e).

Hop matmuls run in fp8(e4m3) with perf_mode=DoubleRow (w_vi scaled by 256
on host; compensated via the tanh activation's scale input). l1 stays
bf16 (fp8 there pushes rel err to ~1.4e-2, too close to the gate).

Host-side (untimed) packing puts every tensor in exact SBUF layout:
  vi   [pair, p, ct, 392]    bf16
  vq   [p, ht, b, t]         bf16
  w1   [p, ct, m]            bf16  (= l1_w.T tiles: c = ct*128+p, h = m)
  wvi* [p, ht, k]            f8    (= w_vi.T * 256)
  wu*  [p, ht, k]            bf16  (= w_u.T)
  l1b  [p, ht] f32, bu* [p, kt] f32
  out  [p, kt, b]            f32   (u transposed; host untransposes)
"""

import numpy as np
from ml_dtypes import bfloat16, float8_e4m3

import concourse.bass as bass
import concourse.tile as tile
from concourse import bacc, mybir
from concourse.bass import ts, ds
from concourse.bass_utils import run_bass_kernel_spmd

BF = mybir.dt.bfloat16
F8 = mybir.dt.float8e4
F32 = mybir.dt.float32

NCORES = 8
B = 128
C = 2048
S = 196
HID = 1024
T = 20
BL = B // NCORES
NPAIR = BL // 2
CT = C // 128
HT = HID // 128
S2 = 2 * S
XCH = 4                    # x DMA chunks per pair
CTC = CT // XCH            # ct per chunk

USE_FP8_HOPS = True
WV_SCALE = 256.0

_NC = None


def _build():
    nc = bacc.Bacc(None)

    wvi_dt = F8 if USE_FP8_HOPS else BF

    vi_p = nc.declare_dram_parameter("vi", [NPAIR, XCH, 128, CTC, S2], BF, isOutput=False)
    vq_p = nc.declare_dram_parameter("vq", [128, HT, BL, T], BF, isOutput=False)
    w1_p = nc.declare_dram_parameter("w1", [XCH, 128, CTC, HID], BF, isOutput=False)
    wvi0_p = nc.declare_dram_parameter("wvi0", [128, HT, HID], wvi_dt, isOutput=False)
    wu0_p = nc.declare_dram_parameter("wu0", [128, HT, HID], BF, isOutput=False)
    wvi1_p = nc.declare_dram_parameter("wvi1", [128, HT, HID], wvi_dt, isOutput=False)
    wu1_p = nc.declare_dram_parameter("wu1", [128, HT, HID], BF, isOutput=False)
    l1b_p = nc.declare_dram_parameter("l1b", [128, HT], F32, isOutput=False)
    bu0_p = nc.declare_dram_parameter("bu0", [128, HT], F32, isOutput=False)
    bu1_p = nc.declare_dram_parameter("bu1", [128, HT], F32, isOutput=False)
    out_p = nc.declare_dram_parameter("out", [128, HT, BL], F32, isOutput=True)

    wvi_p = [wvi0_p, wvi1_p]
    wu_p = [wu0_p, wu1_p]
    bu_p = [bu0_p, bu1_p]

    Tanh = mybir.ActivationFunctionType.Tanh
    Exp = mybir.ActivationFunctionType.Exp
    X = mybir.AxisListType.X
    hop_scale = 1.0 / WV_SCALE if USE_FP8_HOPS else 1.0

    with tile.TileContext(nc) as tc:
        with (
            tc.tile_pool(name="weights", bufs=1) as wpool,
            tc.tile_pool(name="xin", bufs=4) as xpool,
            tc.tile_pool(name="vis", bufs=1) as vipool,
            tc.tile_pool(name="small", bufs=1) as spool,
            tc.tile_pool(name="uu", bufs=3) as upool,
            tc.tile_pool(name="act", bufs=4) as apool,
            tc.tile_pool(name="mm", bufs=4, space="PSUM") as mmpool,
            tc.tile_pool(name="vqtp", bufs=2, space="PSUM") as vqtpool,
        ):
            # ---- input loads.  sync queue: vq + per-pair x chunks (stream).
            # gpsimd queue: all weights (w1 first, hop weights later). ----
            vq_sb = wpool.tile([128, HT, BL, T], BF, tag="vq")
            nc.sync.dma_start(out=vq_sb[:], in_=vq_p[:])
            l1b_sb = wpool.tile([128, HT], F32, tag="l1b")
            nc.sync.dma_start(out=l1b_sb[:], in_=l1b_p[:])

            w1_sb = []
            for i in range(XCH):
                w1c = wpool.tile([128, CTC, HID], BF, tag=f"w1c{i}")
                nc.gpsimd.dma_start(out=w1c[:], in_=w1_p[i])
                w1_sb.append(w1c)
            wu_sb = []
            bu_sb = []
            wvi_sb = []
            for i in range(2):
                wu = wpool.tile([128, HT, HID], BF, tag=f"wu{i}")
                nc.gpsimd.dma_start(out=wu[:], in_=wu_p[i][:])
                wu_sb.append(wu)
                bu = wpool.tile([128, HT], F32, tag=f"bu{i}")
                nc.gpsimd.dma_start(out=bu[:], in_=bu_p[i][:])
                bu_sb.append(bu)
                wv = wpool.tile([128, HT, HID], wvi_dt, tag=f"wvi{i}")
                nc.gpsimd.dma_start(out=wv[:], in_=wvi_p[i][:])
                wvi_sb.append(wv)

            # ---- u0 = mean_t(v_q) ----
            u0 = upool.tile([128, HT, BL], F32, tag="u")
            for ht in range(HT):
                nc.vector.reduce_sum(out=u0[:, ht, :], in_=vq_sb[:, ht, :, :], axis=X)
            nc.vector.tensor_scalar_mul(out=u0[:], in0=u0[:], scalar1=1.0 / T)
            ubf0 = spool.tile([128, HT, BL], BF, tag="ubf")
            nc.vector.tensor_copy(out=ubf0[:], in_=u0[:])

            vi_bf = vipool.tile([128, HT, NPAIR, S2], BF, tag="vi")
            if USE_FP8_HOPS:
                vi8 = vipool.tile([128, HT, NPAIR, S2], F8, tag="vi8", name="vi8")
            else:
                vi8 = vi_bf

            r_sb = [spool.tile([128, HT, BL], F32, tag=f"r{h}", name=f"r{h}") for h in range(2)]
            z_sb = [spool.tile([128, HT, BL], F32, tag=f"z{h}", name=f"z{h}") for h in range(2)]
            vqt_sb = [None, None]

            def emit_vqt(hop, ubf):
                """v_q_t[k,b] = w_u[k,:] @ u[:,b] + b_u[k], batched over BL."""
                vqt_ps = vqtpool.tile([128, HT, BL], F32, tag="vqt")
                for kt in range(HT):
                    for ht in range(HT):
                        nc.tensor.matmul(
                            vqt_ps[:, kt, :],
                            wu_sb[hop][:, ht, ts(kt, 128)],
                            ubf[:, ht, :],
                            start=(ht == 0),
                            stop=(ht == HT - 1),
                        )
                v = spool.tile([128, HT, BL], F32, tag=f"vqts{hop}", name=f"vqts{hop}")
                for kt in range(HT):
                    nc.vector.tensor_scalar(
                        out=v[:, kt, :],
                        in0=vqt_ps[:, kt, :],
                        scalar1=bu_sb[hop][:, kt : kt + 1],
                        scalar2=None,
                        op0=mybir.AluOpType.add,
                    )
                vqt_sb[hop] = v

            def emit_hop_pair(hop, pair):
                """One pair's attention hop: logits, exp, weighted sums."""
                ha = apool.tile([128, HT, S2], BF, tag="scr")
                for kt in range(HT):
                    ps = mmpool.tile([128, S2], F32, tag="mm")
                    if USE_FP8_HOPS:
                        for h2 in range(HT // 2):
                            nc.tensor.matmul(
                                ps[:],
                                wvi_sb[hop][:, 2 * h2 : 2 * h2 + 2, ts(kt, 128)],
                                vi8[:, 2 * h2 : 2 * h2 + 2, pair, :],
                                perf_mode=mybir.MatmulPerfMode.DoubleRow,
                                start=(h2 == 0),
                                stop=(h2 == HT // 2 - 1),
                            )
                    else:
                        for ht in range(HT):
                            nc.tensor.matmul(
                                ps[:],
                                wvi_sb[hop][:, ht, ts(kt, 128)],
                                vi_bf[:, ht, pair, :],
                                start=(ht == 0),
                                stop=(ht == HT - 1),
                            )
                    for j in range(2):
                        b = 2 * pair + j
                        nc.scalar.activation(
                            out=ha[:, kt, ds(S * j, S)],
                            in_=ps[:, ds(S * j, S)],
                            func=Tanh,
                            bias=vqt_sb[hop][:, kt, b : b + 1],
                            scale=hop_scale,
                        )
                e = apool.tile([128, HT, S2], BF, tag="scr")
                nc.scalar.activation(out=e[:], in_=ha[:], func=Exp)
                nc.vector.reduce_sum(
                    out=z_sb[hop][:, :, ts(pair, 2)],
                    in_=e[:].rearrange("p h (j s) -> p h j s", j=2),
                    axis=X,
                )
                tt = apool.tile([128, HT, S2], BF, tag="scr")
                nc.vector.tensor_mul(out=tt[:], in0=e[:], in1=vi_bf[:, :, pair, :])
                nc.vector.reduce_sum(
                    out=r_sb[hop][:, :, ts(pair, 2)],
                    in_=tt[:].rearrange("p h (j s) -> p h j s", j=2),
                    axis=X,
                )

            def emit_u_update(hop, u_prev):
                zr = spool.tile([128, HT, BL], F32, tag=f"zr{hop}")
                nc.vector.reciprocal(out=zr[:], in_=z_sb[hop][:])
                upd = spool.tile([128, HT, BL], F32, tag=f"upd{hop}")
                nc.vector.tensor_mul(out=upd[:], in0=r_sb[hop][:], in1=zr[:])
                u_next = upool.tile([128, HT, BL], F32, tag="u")
                nc.vector.tensor_add(out=u_next[:], in0=u_prev[:], in1=upd[:])
                return u_next

            # ---- phase A: interleaved l1 + hop0 per pair ----
            for pair in range(NPAIR):
                xc = []
                for i in range(XCH):
                    x_sb = xpool.tile([128, CTC, S2], BF, tag="x")
                    nc.sync.dma_start(out=x_sb[:], in_=vi_p[pair, i])
                    xc.append(x_sb)
                for ht in range(HT):
                    ps = mmpool.tile([128, S2], F32, tag="mm")
                    for ct in range(CT):
                        nc.tensor.matmul(
                            ps[:],
                            w1_sb[ct // CTC][:, ct % CTC, ts(ht, 128)],
                            xc[ct // CTC][:, ct % CTC, :],
                            start=(ct == 0),
                            stop=(ct == CT - 1),
                        )
                    nc.scalar.activation(
                        out=vi_bf[:, ht, pair, :],
                        in_=ps[:],
                        func=Tanh,
                        bias=l1b_sb[:, ht : ht + 1],
                        scale=1.0,
                    )
                    if USE_FP8_HOPS:
                        nc.vector.tensor_copy(
                            out=vi8[:, ht, pair, :], in_=vi_bf[:, ht, pair, :]
                        )
                if pair == 0:
                    emit_vqt(0, ubf0)
                emit_hop_pair(0, pair)

            # ---- boundary: u1 and hop1's v_q_t (batched) ----
            u1 = emit_u_update(0, u0)
            ubf1 = spool.tile([128, HT, BL], BF, tag="ubf")
            nc.vector.tensor_copy(out=ubf1[:], in_=u1[:])
            emit_vqt(1, ubf1)

            # ---- phase B: hop1 for all pairs ----
            for pair in range(NPAIR):
                emit_hop_pair(1, pair)

            u2 = emit_u_update(1, u1)
            nc.sync.dma_start(out=out_p[:], in_=u2[:])

    nc.compile()
    return nc


def _get_nc():
    global _NC
    if _NC is None:
        _NC = _build()
    return _NC


def _prep_in_maps(v_i, v_q, l1_w, l1_b, w_vi0, w_u0, b_u0, w_vi1, w_u1, b_u1):
    v_i = np.asarray(v_i, np.float32)
    v_q = np.asarray(v_q, np.float32)

    # vi: [B, C, H, W] -> [core, pair, p, ct, j, s] -> [core, pair, xch, p, ctc, 392]
    vib = v_i.reshape(B, C, S).astype(bfloat16)
    vib = vib.reshape(NCORES, NPAIR, 2, CT, 128, S).transpose(0, 1, 4, 3, 2, 5)
    vib = np.ascontiguousarray(vib).reshape(NCORES, NPAIR, 128, XCH, CTC, S2)
    vib = np.ascontiguousarray(vib.transpose(0, 1, 3, 2, 4, 5))

    # vq: [B, T, HID] -> [core, p, ht, b, t]
    vq = v_q.reshape(NCORES, BL, T, HT, 128).transpose(0, 4, 3, 1, 2)
    vq = np.ascontiguousarray(vq.astype(bfloat16))

    def packT(w, ntiles, dt, scale=1.0):
        wt = (np.asarray(w, np.float32).T * scale).astype(dt)
        return np.ascontiguousarray(
            wt.reshape(ntiles, 128, w.shape[0]).transpose(1, 0, 2)
        )

    # w1 packed as [xch, p, ctc, m]
    w1h = packT(l1_w, CT, bfloat16)  # [128, 16, 1024]
    w1h = np.ascontiguousarray(
        w1h.reshape(128, XCH, CTC, HID).transpose(1, 0, 2, 3)
    )

    wvi_dt = float8_e4m3 if USE_FP8_HOPS else bfloat16
    wvi_s = WV_SCALE if USE_FP8_HOPS else 1.0
    wvi0h = packT(w_vi0, HT, wvi_dt, wvi_s)
    wvi1h = packT(w_vi1, HT, wvi_dt, wvi_s)
    wu0h = packT(w_u0, HT, bfloat16)
    wu1h = packT(w_u1, HT, bfloat16)

    def packb(b):
        return np.ascontiguousarray(np.asarray(b, np.float32).reshape(HT, 128).T)

    l1bh = packb(l1_b)
    bu0h = packb(b_u0)
    bu1h = packb(b_u1)

    in_maps = []
    for core in range(NCORES):
        in_maps.append(
            {
                "vi": vib[core],
                "vq": vq[core],
                "w1": w1h,
                "wvi0": wvi0h,
                "wu0": wu0h,
                "wvi1": wvi1h,
                "wu1": wu1h,
                "l1b": l1bh,
                "bu0": bu0h,
                "bu1": bu1h,
            }
        )
    return in_maps


def run_sharded(inputs: dict, trace: bool = False):
    """Returns (full_output [128,1024] f32, BassKernelResults)."""
    nc = _get_nc()
    in_maps = _prep_in_maps(**inputs)
    res = run_bass_kernel_spmd(
        nc, in_maps, core_ids=list(range(NCORES)), trace=trace
    )
    outs = []
    for i in range(NCORES):
        o = np.asarray(res.results[i]["out"])  # [128, HT, BL] = [p, ht, b]
        outs.append(np.ascontiguousarray(o.transpose(2, 1, 0)).reshape(BL, HID))
    full = np.concatenate(outs, axis=0).astype(np.float32)
    return full, res


def kernel(**inputs) -> np.ndarray:
    out, _ = run_sharded(inputs, trace=False)
    return out
